# revision 53
# baseline (speedup 1.0000x reference)
"""Blocksparse dilated attention TRN2 kernel.

Sharding: 8 cores = r(=4 dilation offsets) x B(=2 batch). Each core runs one
independent per-offset attention branch on its strided token subset
(x[b, o::r, :]), with that offset's own weights. Host does the strided
gather (+pack to the kernel's DMA-friendly layouts) and the final scatter
into the zero-padded (B, S, r*D) output.

Per-core math (L=2048 tokens, D=768, H=12 heads, hd=64, segment=512):
  qkvT = Wqkv @ xoT            (channel-on-partition for q,k; token-major v)
  per (segment, head):  scoresT = kT-chunks.T x qT   (k on partitions)
                        attnT = exp(scale * scoresT)  (no max-subtract:
                              scores std ~0.3, max ~1.5 -> exp safe in fp32)
                        [ctxu; denom] = [v | ones].T @ attnT  (ones column
                              makes PSUM row 64 the softmax denominator)
  per (segment, chunk): rc = 1/denom  (DVE reciprocal after a DMA spread
                              across 128 partitions)
                        ctxT = ctxu * broadcast(rc)  (rc staged to DRAM,
                              then partition-step-0 broadcast DMA per chunk)
  outT = Wout @ ctxT + bout

Matmuls run in bf16 (full PE rate; fp32 PSUM accumulation). fp32/fp32r
matmuls are unusable here (walrus single sync-wait slot on fused
load+matmul); fp8 was measured on CPU at rel_err 1.9e-2 even in the
safest (q,k-proj only) variant -- too close to the 2e-2 gate.

Scheduling notes (each earned from a perfetto trace):
- Emission order software-pipelines segments so the PE never idles long
  enough (~3.4us) for the HAM clock gate to drop it from 2.4 to 1.2 GHz:
  proj(s) -> [normalize(s-1) + outproj(s-1)] -> attention(s).
- The reciprocal chain (den-copy -> spread-DMA -> DVE recip -> writeback)
  is latency-bound (~2us per DMA hop). Its stages are emitted STAGGERED
  one chunk apart so every queue item's wait is ~zero when it reaches its
  engine's head: an in-order engine queue head-of-line-blocks on any long
  wait (this serialized the whole tail when all stages sat adjacent on
  the gpsimd queue).
- For segments whose normalize runs a full segment later, the reciprocal
  rows go to DRAM and come back via a gpsimd SWDGE partition-step-0
  broadcast DMA (the only engine allowed such a source AP). For the FINAL
  segment that path is unusable: SWDGE broadcast completion latency was
  measured at 5-16us. Instead the spread reciprocal is linearized by one
  small SBUF->SBUF DMA into [2, SEG] rows and broadcast across the 128
  partitions by a K=2 PE matmul against a host-provided block-selection
  mask (sel2), with the ensuing multiply reading the PSUM product.
- The final tail keeps the PE busy (and HAM warm) while the last chunks'
  chains drain: segment SL-1's outproj columns 4,5 are withheld from the
  filler budget and emitted in the tail, plus a 2-bank dc-progressive
  accumulation for SL's first two outproj columns.
- Weights/xo are DMA'd from HOST-PACKED buffers: each descriptor reads
  DRAM fully sequentially (a strided [p, dc, e] gather out of the natural
  [D, E3] layout measured ~53 GB/s; packed runs near peak, but a single
  HWDGE queue still only sustains ~90 GB/s). The qk weight stream issues
  on sync and xo/v/bias bytes on scalar so the two streams transfer in
  parallel during the cold start; output stores also ride scalar so they
  never head-of-line block the tail's small latency-critical sync DMAs.
"""

import math
import sys
from contextlib import ExitStack

import ml_dtypes
import numpy as np

for _p in ("/opt/trn_rl_repo",):
    if _p not in sys.path:
        sys.path.insert(0, _p)

import concourse.bass as bass
import concourse.mybir as mybir
import concourse.tile as tile
from concourse import bacc
from concourse.bass_utils import run_bass_kernel_spmd

P = 128

# Problem constants (hardcoded per harness contract)
B0, S0, D0 = 2, 8192, 768
R0 = 4
H0, HD0 = 12, 64
SEG0 = 512
NSEG0 = (S0 // R0) // SEG0  # 4
N_CORES = 8

F32 = mybir.dt.float32
BF16 = mybir.dt.bfloat16

# qk weight stream descriptor groups (dc0, ndc, col0, ncols within the 2*D
# qk sections): the first groups gate the very first matmul groups so they
# get small dc-split descriptors; the rest come in column pairs.
DC0 = D0 // P
QK_GROUPS = ([(0, 3, 0, P), (3, 3, 0, P), (0, DC0, P, P)]
             + [(0, DC0, c, 2 * P) for c in range(2 * P, 2 * D0, 2 * P)])
V_GROUPS = [(0, DC0, 0, 512), (0, DC0, 512, D0 - 512)]  # cols within v section
# segment-0 xo split: (dc0) tiny first on scalar to unblock the first MM,
# (dc1-2) on scalar, (dc3-5) interleaved into the sync weight stream
XO0_GROUPS = [(0, 1), (1, 2), (3, 3)]


def build_nc(D=D0, H=H0, HD=HD0, SEG=SEG0, NSEG=NSEG0, mm_dt=BF16):
    """Build the per-core Bass program (same NEFF on all cores)."""
    DC = D // P                # channel chunks of 128
    L = SEG * NSEG             # tokens per core
    KC = SEG // P              # key chunks per segment
    HPC = P // HD              # heads per 128-channel chunk
    E3 = 3 * D
    HV = HD + 1                # v columns per head incl. ones column
    scale = 1.0 / math.sqrt(HD)
    assert D == H * HD and SEG % P == 0 and D % P == 0 and KC % 2 == 0

    nc = bacc.Bacc(trn_type="TRN2")
    xo_pk = nc.dram_tensor("xo_pk", [NSEG, D * SEG], mm_dt, kind="ExternalInput")
    wqk_pk = nc.dram_tensor("wqk_pk", [2 * D * D], mm_dt, kind="ExternalInput")
    wv_pk = nc.dram_tensor("wv_pk", [D * D], mm_dt, kind="ExternalInput")
    wout_pk = nc.dram_tensor("wout_pk", [D * D], mm_dt, kind="ExternalInput")
    bqkv_pt = nc.dram_tensor("bqkv_pt", [P, 3 * DC], F32, kind="ExternalInput")
    bout_pt = nc.dram_tensor("bout_pt", [P, DC], F32, kind="ExternalInput")
    bv = nc.dram_tensor("bv", [D], F32, kind="ExternalInput")
    sel2_d = nc.dram_tensor("sel2_d", [P // HD, P], mm_dt, kind="ExternalInput")
    out_pk = nc.dram_tensor("out_pk", [NSEG, D * SEG], F32, kind="ExternalOutput")
    # scratch for the softmax reciprocals: broadcast-DMA needs a DRAM source
    # (SBUF-source partition-step-0 APs are rejected)
    rc_dram = nc.dram_tensor("rc_dram", [NSEG, H * SEG], mm_dt, kind="Internal")

    def packed_ap(t, flat_off, ncols, ndc=None):
        """AP over a host-packed [P, ndc, ncols] block starting at flat_off.
        The element stream is fully sequential in DRAM."""
        if ndc is None:
            ndc = DC
        base = t[:]
        return bass.AP(tensor=base.tensor, offset=flat_off,
                       ap=[[ndc * ncols, P], [ncols, ndc], [1, ncols]])

    with ExitStack() as ctx:
        tc = ctx.enter_context(tile.TileContext(nc))
        singles = ctx.enter_context(tc.tile_pool(name="singles", bufs=1))
        xo_pool = ctx.enter_context(tc.tile_pool(name="xo", bufs=2))
        qk_pool = ctx.enter_context(tc.tile_pool(name="qk", bufs=2))
        v_pool = ctx.enter_context(tc.tile_pool(name="v", bufs=2))
        attn_pool = ctx.enter_context(tc.tile_pool(name="attn", bufs=3))
        ctxu_pool = ctx.enter_context(tc.tile_pool(name="ctxu", bufs=2))
        den_pool = ctx.enter_context(tc.tile_pool(name="den", bufs=1))
        rc_pool = ctx.enter_context(tc.tile_pool(name="rc", bufs=2))
        rows_pool = ctx.enter_context(tc.tile_pool(name="rcrows", bufs=6))
        ctxs_pool = ctx.enter_context(tc.tile_pool(name="ctxs", bufs=2))
        out_pool = ctx.enter_context(tc.tile_pool(name="outp", bufs=2))
        bcast_pool = ctx.enter_context(tc.tile_pool(name="bcast", bufs=6))
        pp_proj = ctx.enter_context(tc.tile_pool(name="pp_proj", bufs=2, space="PSUM"))
        pp_scA = ctx.enter_context(tc.tile_pool(name="pp_scA", bufs=1, space="PSUM"))
        pp_scB = ctx.enter_context(tc.tile_pool(name="pp_scB", bufs=1, space="PSUM"))
        pp_cb = ctx.enter_context(tc.tile_pool(name="pp_cb", bufs=2, space="PSUM"))

        # --- segment-0 xo halves, one per HWDGE queue (a single queue only
        # sustains ~90 GB/s, so the two descriptor streams transfer the
        # cold-start-critical bytes in parallel) ---
        xo_first = xo_pool.tile([P, DC, SEG], mm_dt, tag="xo", name="xo_s0")
        xo0_off = {}
        off = 0
        for dc0, ndc in XO0_GROUPS:
            xo0_off[dc0] = (off, ndc)
            off += P * ndc * SEG

        def xo0_dma(eng, dc0):
            o, ndc = xo0_off[dc0]
            eng.dma_start(out=xo_first[:, dc0:dc0 + ndc, :],
                          in_=packed_ap(xo_pk, o, SEG, ndc))

        xo0_dma(nc.scalar, 0)        # 128 KB: unblocks the very first MM
        xo0_dma(nc.scalar, 1)

        # tiny biases (a late bias DMA stalls the first qk-add on DVE and
        # cascades into a PE psum-WAR stall)
        bqkv_sb = singles.tile([P, 3 * DC], F32, tag="bqkv")
        nc.scalar.dma_start(out=bqkv_sb, in_=bqkv_pt[:, :])
        bout_sb = singles.tile([P, DC], F32, tag="bout")
        nc.scalar.dma_start(out=bout_sb, in_=bout_pt[:, :])
        # v-section bias broadcast along partitions (natural layout add)
        bv_sb = singles.tile([P, D], F32, tag="bv")
        bv_ap = bv[:]
        bv_bcast = bass.AP(tensor=bv_ap.tensor, offset=bv_ap.offset,
                           ap=[[0, P], *bv_ap.ap])
        nc.gpsimd.dma_start(out=bv_sb, in_=bv_bcast)

        # qk weight stream on sync, in first-use (column-group) order, with
        # the second xo0 half interleaved after the first weight group
        w_qkv_sb = singles.tile([P, DC, E3], mm_dt, tag="wqkv")
        off = 0
        for gi, (dc0, ndc, c0, nco) in enumerate(QK_GROUPS):
            nc.sync.dma_start(out=w_qkv_sb[:, dc0:dc0 + ndc, c0:c0 + nco],
                              in_=packed_ap(wqk_pk, off, nco, ndc))
            off += P * ndc * nco
            if gi == 1:
                xo0_dma(nc.sync, 3)
        off = 0
        for dc0, ndc, c0, nco in V_GROUPS:
            nc.scalar.dma_start(out=w_qkv_sb[:, dc0:dc0 + ndc,
                                             2 * D + c0:2 * D + c0 + nco],
                                in_=packed_ap(wv_pk, off, nco, ndc))
            off += P * ndc * nco
        # wout on the (otherwise idle) gpsimd queue: it is needed last and
        # this keeps the scalar queue free for xo/v-weight bytes
        w_out_sb = singles.tile([P, DC, D], mm_dt, tag="wout")
        nc.gpsimd.dma_start(out=w_out_sb[:, :, :], in_=packed_ap(wout_pk, 0, D))

        # 2x128 block-selection mask for the tail's PE-broadcast of the
        # reciprocal rows (sel2[h, m] = 1 iff m//HD == h); host-provided
        # (single-partition memsets at partition base 1 fail BIR verification)
        sel2 = singles.tile([HPC, P], mm_dt, tag="sel2")
        nc.scalar.dma_start(out=sel2, in_=sel2_d[:, :])

        def load_and_proj(s, pre_tasks=()):
            """xo load + qkv projections for segment s. pre_tasks are emitted
            after the first two qk chunks (prev segment's last recip chain --
            by then its dent DMA has landed)."""
            st = {}
            if s == 0:
                xo_s = xo_first
            else:
                xo_s = xo_pool.tile([P, DC, SEG], mm_dt, tag="xo", name=f"xo_s{s}")
                nc.scalar.dma_start(out=xo_s[:, :, :],
                                    in_=packed_ap(xo_pk, s * D * SEG, SEG))
            st["xo"] = xo_s
            st["ctxu"] = ctxu_pool.tile([P, DC, SEG], mm_dt, tag="ctxu",
                                        name=f"ctxu{s}")
            st["den"] = den_pool.tile([1, H * SEG], F32, tag="den",
                                      name=f"den{s}")
            st["ctx_s"] = ctxs_pool.tile([P, DC, SEG], mm_dt, tag="ctxs",
                                         name=f"cs{s}")
            st["out_seg"] = out_pool.tile([P, DC, SEG], F32, tag="ot",
                                          name=f"ot{s}")

            # q,k in transposed layout (e on partitions)
            qk_s = qk_pool.tile([P, 2 * DC, SEG], mm_dt, tag="qk", name=f"qk_s{s}")
            st["qk"] = qk_s
            for ec in range(2 * DC):
                ps = pp_proj.tile([P, SEG], F32, tag="proj", name=f"psqk{s}_{ec}")
                for dc in range(DC):
                    nc.tensor.matmul(
                        ps,
                        w_qkv_sb[:, dc, ec * P:(ec + 1) * P],
                        xo_s[:, dc, :],
                        start=(dc == 0), stop=(dc == DC - 1))
                nc.vector.tensor_scalar_add(qk_s[:, ec, :], ps, bqkv_sb[:, ec:ec + 1])
                if ec == 1:
                    for t in pre_tasks:
                        t()

            # v in natural layout (token on partitions), per-head + ones column
            v_s = v_pool.tile([P, KC, H * HV], mm_dt, tag="v", name=f"v_s{s}")
            st["v"] = v_s
            v_view = v_s.rearrange("p k (h c) -> p k h c", c=HV)
            nc.vector.memset(v_view[:, :, :, HD:HD + 1], 1.0)
            for lc in range(KC):
                for n0 in range(0, D, 512):
                    n = min(512, D - n0)
                    nh = n // HD
                    h0 = n0 // HD
                    psv = pp_proj.tile([P, SEG], F32, tag="proj",
                                       name=f"psv{s}_{lc}_{n0}")
                    for dc in range(DC):
                        nc.tensor.matmul(
                            psv[:, :n],
                            xo_s[:, dc, lc * P:(lc + 1) * P],
                            w_qkv_sb[:, dc, 2 * D + n0: 2 * D + n0 + n],
                            start=(dc == 0), stop=(dc == DC - 1))
                    nc.vector.tensor_add(
                        v_view[:, lc, h0:h0 + nh, 0:HD],
                        psv[:, :n].rearrange("p (h c) -> p h c", c=HD),
                        bv_sb[:, n0:n0 + n].rearrange("p (h c) -> p h c", c=HD))
            return st

        def dent_dma(s, st, hc0, nhc, tag):
            """Spread den elements for chunks [hc0,hc0+nhc) over 128
            partitions (DVE reciprocal is ~8 cyc/elem/lane; a 1-partition
            strip would take ~6us; element order is irrelevant since the
            writeback DMA restores it)."""
            den = st["den"]
            e0, ne = hc0 * HPC * SEG, nhc * HPC * SEG
            den_t = rc_pool.tile([P, ne // P], F32, tag=f"dent{tag}",
                                 name=f"dent{s}_{hc0}")
            nc.sync.dma_start(out=den_t, in_=den[0:1, e0:e0 + ne])
            st[f"dent_{hc0}_{nhc}"] = den_t

        def recip_writeback(s, st, hc0, nhc, tag, to_rows=False):
            """DVE reciprocal of the spread denominators, then either write
            back to DRAM (for the SWDGE broadcast path) or, for the tail
            chunks, a single SBUF->SBUF linearizing DMA into a [HPC, SEG]
            row tile consumed by the PE-broadcast (the SWDGE broadcast has
            ~5-7us completion latency that the tail cannot hide)."""
            den_t = st.pop(f"dent_{hc0}_{nhc}")
            e0, ne = hc0 * HPC * SEG, nhc * HPC * SEG
            rc_t = rc_pool.tile([P, ne // P], mm_dt, tag=f"rct{tag}",
                                name=f"rct{s}_{hc0}")
            with nc.allow_low_precision(
                    reason="softmax denominator reciprocal; bf16 scale factor"):
                nc.vector.reciprocal(rc_t, den_t)
            if to_rows:
                rows = rows_pool.tile([HPC, SEG], mm_dt, tag="rcrows",
                                      name=f"rcrows{s}_{hc0}")
                nc.sync.dma_start(out=rows, in_=rc_t[:, :])
                st.setdefault("rcrows", {})[hc0] = rows
            else:
                nc.sync.dma_start(out=rc_dram[s:s + 1, e0:e0 + ne], in_=rc_t)

        def attention(s, st, filler=(), per_chunk_recip=False):
            """scores + exp + unnormalized ctx (and denom), processed in
            head PAIRS: the two heads of a 128-channel chunk occupy PE
            row-groups 0-63 and 64-127, and their K=64 scores matmuls are
            emitted adjacently so the array runs them concurrently (~2x on
            the scores phase). Pipelined: ctx(pair-1) after scores(pair).
            `filler` tasks (prev segment's normalize + outproj) are emitted
            between pairs so the PE has work while ACT exp catches up.
            Returns deferred end-of-segment recip tasks for the caller to
            emit inside the next segment's projection."""
            filler = list(filler)
            n_filler = len(filler)
            emitted = 0
            qk_s, v_s = st["qk"], st["v"]
            ctxu, den = st["ctxu"], st["den"]
            ats = {}
            HH = DC // 2
            for c in range(DC + 1):
                while emitted < (c * n_filler) // DC:
                    filler[emitted]()
                    emitted += 1
                if c < DC:
                    at2 = attn_pool.tile([P, HPC, KC, SEG], mm_dt, tag="attn",
                                         name=f"at{s}_{c}")
                    ats[c] = at2
                    for w in range(KC // 2):
                        for half, pool in ((0, pp_scA), (1, pp_scB)):
                            kc = 2 * w + half
                            sc = pool.tile([P, HPC, SEG], F32, tag=f"sc{half}",
                                           name=f"sc{half}_{s}_{c}_{w}")
                            for i in range(HPC):
                                ho = i * HD
                                nc.tensor.matmul(
                                    sc[:, i, :],
                                    qk_s[ho:ho + HD, DC + c, kc * P:(kc + 1) * P],
                                    qk_s[ho:ho + HD, c, :])
                            nc.scalar.activation(
                                at2[:, :, kc, :], sc,
                                mybir.ActivationFunctionType.Exp,
                                scale=scale)
                if c > 0:
                    cp = c - 1
                    at2 = ats.pop(cp)
                    for i in range(HPC):
                        h = cp * HPC + i
                        ho = i * HD
                        cps = pp_cb.tile([P, SEG], F32, tag="cb",
                                         name=f"cps{s}_{h}")
                        for kc in range(KC):
                            nc.tensor.matmul(
                                cps[0:HD + 1, :],
                                v_s[:, kc, h * HV:(h + 1) * HV],
                                at2[:, i, kc, :],
                                start=(kc == 0), stop=(kc == KC - 1))
                        # den copy first: it is on the reciprocal-chain
                        # critical path, the ctxu cast is not
                        nc.vector.tensor_copy(den[0:1, h * SEG:(h + 1) * SEG],
                                              cps[HD:HD + 1, :])
                        nc.vector.tensor_copy(ctxu[ho:ho + HD, cp, :],
                                              cps[0:HD, :])
                    # staggered reciprocal chains: each stage is emitted a
                    # chunk after its producer so no engine-queue head ever
                    # waits long (HOL blocking)
                    if per_chunk_recip:
                        # all chunks of the final segment use the PE-broadcast
                        # rows path: SWDGE broadcast completion latency was
                        # measured at 5-16us and stalls the tail's DVE queue
                        dent_dma(s, st, cp, 1, "c")
                        if cp >= 1:
                            recip_writeback(s, st, cp - 1, 1, "c",
                                            to_rows=True)
                    else:
                        if cp == HH - 1:
                            dent_dma(s, st, 0, HH, "h")
                        elif cp == HH + 1:
                            recip_writeback(s, st, 0, HH, "h")
                            # pre-issue the SWDGE broadcasts now: their
                            # 5-16us completion latency must be paid long
                            # before the next segment's normalize fillers
                            for hc in range(HH):
                                bcast_chunk(s, st, hc)
                        elif cp == DC - 1:
                            dent_dma(s, st, HH, DC - HH, "h")
            if per_chunk_recip:
                return [lambda: recip_writeback(s, st, DC - 1, 1, "c",
                                                to_rows=True)]

            def second_half():
                recip_writeback(s, st, HH, DC - HH, "h")
                for hc in range(HH, DC):
                    bcast_chunk(s, st, hc)
            return [second_half]

        def bcast_chunk(s, st, hc):
            """Broadcast the 2 per-head reciprocal rows of chunk hc across HD
            partitions by an SWDGE DMA (partition-step-0 source AP) instead
            of a PE outer-product."""
            bcs = bcast_pool.tile([P, SEG], mm_dt, tag="bcs",
                                  name=f"bcs{s}_{hc}")
            rr = rc_dram[s:s + 1, hc * HPC * SEG:(hc + 1) * HPC * SEG]
            rr_b = bass.AP(tensor=rr.tensor, offset=rr.offset,
                           ap=[[SEG, HPC], [0, HD], [1, SEG]])
            nc.gpsimd.dma_start(out=bcs, in_=rr_b)
            st.setdefault("bcs", {})[hc] = bcs
            return bcs

        def norm_chunk(s, st, hc):
            """Normalize one 128-channel (2-head) chunk."""
            ctxu, ctx_s = st["ctxu"], st["ctx_s"]
            rows = st.get("rcrows", {}).pop(hc, None)
            if rows is not None:
                # tail path: PE outer-product broadcast (K=2 matmul against
                # the block-selection mask) instead of the slow SWDGE DMA
                bc = pp_cb.tile([P, SEG], F32, tag="cb", name=f"bcps{s}_{hc}")
                nc.tensor.matmul(bc, sel2, rows)
                nc.vector.tensor_mul(ctx_s[:, hc, :], ctxu[:, hc, :], bc)
                return
            bcs = st.get("bcs", {}).pop(hc, None)
            if bcs is None:
                bcs = bcast_chunk(s, st, hc)
                st["bcs"].pop(hc)
            nc.vector.tensor_mul(ctx_s[:, hc, :], ctxu[:, hc, :], bcs)

        def outproj_fc(s, st, fc, pool=None, tag="proj"):
            """Full out-projection column fc (all DC contraction chunks)."""
            ctx_s, out_seg = st["ctx_s"], st["out_seg"]
            pso = (pool or pp_proj).tile([P, SEG], F32, tag=tag,
                                         name=f"pso{s}_{fc}")
            for dc in range(DC):
                nc.tensor.matmul(
                    pso,
                    w_out_sb[:, dc, fc * P:(fc + 1) * P],
                    ctx_s[:, dc, :],
                    start=(dc == 0), stop=(dc == DC - 1))
            nc.vector.tensor_scalar_add(out_seg[:, fc, :], pso,
                                        bout_sb[:, fc:fc + 1])

        def store_out(s, st):
            # out_pk is fc-major [NSEG, DC, P, SEG]; this AP enumerates it
            # in the tile's (p, dc, t) order. Stores ride the scalar queue:
            # a 1.5MB store on sync would head-of-line block the tail's
            # small latency-critical DMAs.
            base = out_pk[:]
            nc.scalar.dma_start(
                out=bass.AP(tensor=base.tensor, offset=s * D * SEG,
                            ap=[[SEG, P], [P * SEG, DC], [1, SEG]]),
                in_=st["out_seg"][:, :, :])

        def store_fc(s, st, fc, eng):
            """Store one out-projection column as soon as its bias-add is
            done (the final segment's single 1.5MB store exposed ~5.5us of
            pure drain after the last matmul)."""
            base = out_pk[:]
            eng.dma_start(
                out=bass.AP(tensor=base.tensor,
                            offset=(s * DC + fc) * P * SEG,
                            ap=[[SEG, P], [1, SEG]]),
                in_=st["out_seg"][:, fc, :])

        def norm_tasks(s, st, nfc=DC, store=True):
            """Deferred normalize + outproj tasks (run as PE/DVE filler inside
            the next segment's attention). nfc/store limit how much outproj
            is spent as filler -- the rest is saved as tail fill."""
            tasks = ([(lambda hc=hc: norm_chunk(s, st, hc)) for hc in range(DC)]
                     + [(lambda fc=fc: outproj_fc(s, st, fc)) for fc in range(nfc)])
            if store:
                tasks.append(lambda: store_out(s, st))
            return tasks

        sts = {}
        SL = NSEG - 1
        deferred = []
        for s in range(NSEG):
            sts[s] = load_and_proj(s, pre_tasks=deferred)
            if s > 0:
                filler = list(norm_tasks(s - 1, sts[s - 1], nfc=4 if s == SL else DC,
                                         store=(s != SL)))
                if s != SL:
                    sts.pop(s - 1)
            else:
                filler = []
            deferred = attention(s, sts[s], filler, per_chunk_recip=(s == SL))

        # --- final segment tail. The reciprocal chains of the last two
        # chunks have ~8us of unavoidable DMA latency; the saved outproj
        # work of segment SL-1 (fc 4,5) plus a 2-bank dc-progressive for
        # SL's fc 0,1 keeps the PE busy (and HAM warm) while they drain;
        # fc 2-5 of SL run as ordinary full groups at the end. ---
        stp, stl = sts.pop(SL - 1), sts.pop(SL)
        ctx_s, out_seg = stl["ctx_s"], stl["out_seg"]
        norm_chunk(SL, stl, 0)
        norm_chunk(SL, stl, 1)
        for t in deferred:
            t()
        outproj_fc(SL - 1, stp, 4)
        outproj_fc(SL - 1, stp, 5)
        store_out(SL - 1, stp)
        n_prog = 2
        psos = {}
        for fc in range(n_prog):
            psos[fc] = pp_proj.tile([P, SEG], F32, tag="proj", name=f"psoL_{fc}")
        for dc in range(DC):
            if dc >= 2:
                norm_chunk(SL, stl, dc)
            for fc in range(n_prog):
                nc.tensor.matmul(
                    psos[fc],
                    w_out_sb[:, dc, fc * P:(fc + 1) * P],
                    ctx_s[:, dc, :],
                    start=(dc == 0), stop=(dc == DC - 1))
        for fc in range(n_prog):
            nc.vector.tensor_scalar_add(out_seg[:, fc, :], psos[fc],
                                        bout_sb[:, fc:fc + 1])
            store_fc(SL, stl, fc, nc.scalar if fc % 2 else nc.sync)
        for fc in range(n_prog, DC):
            # alternate PSUM pools: a 2-bank rotation stalls each trailing
            # group ~1.3us on the previous group's TS drain (WAR)
            outproj_fc(SL, stl, fc,
                       pool=(pp_cb if fc % 2 else pp_proj),
                       tag=("cb" if fc % 2 else "proj"))
            store_fc(SL, stl, fc, nc.scalar if fc % 2 else nc.sync)

    nc.compile()
    return nc


def _pack_w(wT, c0, ncols, dc0=0, ndc=D0 // P):
    """[D, cols] slice -> packed [P, ndc, ncols] stream (fully sequential)."""
    return np.ascontiguousarray(
        wT[:, c0:c0 + ncols].reshape(D0 // P, P, ncols)[dc0:dc0 + ndc]
        .transpose(1, 0, 2)).ravel()


def make_in_maps(x, Wqkv, bqkv, Wout, bout):
    """Shard full inputs across 8 cores: core = o*B + b."""
    r, E3, D = Wqkv.shape
    Bb, S, _ = x.shape
    DC = D // P
    in_maps = []
    for c in range(r * Bb):
        o, b = c // Bb, c % Bb
        wT = np.ascontiguousarray(Wqkv[o].T).astype(ml_dtypes.bfloat16)
        woT = np.ascontiguousarray(Wout[o].T).astype(ml_dtypes.bfloat16)
        xoT = np.ascontiguousarray(x[b, o::r, :].T).astype(ml_dtypes.bfloat16)
        wqk_pk = np.concatenate([_pack_w(wT, c0, nco, dc0, ndc)
                                 for dc0, ndc, c0, nco in QK_GROUPS])
        wv_pk = np.concatenate([_pack_w(wT, 2 * D + c0, nco, dc0, ndc)
                                for dc0, ndc, c0, nco in V_GROUPS])
        wout_pk = _pack_w(woT, 0, D)
        xo0 = np.concatenate([_pack_w(xoT, 0, SEG0, dc0, ndc)
                              for dc0, ndc in XO0_GROUPS])
        xo_pk = np.stack([xo0] + [_pack_w(xoT, s * SEG0, SEG0)
                                  for s in range(1, NSEG0)])
        in_maps.append({
            "xo_pk": xo_pk,
            "wqk_pk": wqk_pk,
            "wv_pk": wv_pk,
            "wout_pk": wout_pk,
            "bqkv_pt": np.ascontiguousarray(bqkv[o].reshape(3 * DC, P).T),
            "bout_pt": np.ascontiguousarray(bout[o].reshape(DC, P).T),
            "bv": np.ascontiguousarray(bqkv[o, 2 * D:3 * D]),
            "sel2_d": np.ascontiguousarray(
                np.kron(np.eye(P // HD0, dtype=np.float32),
                        np.ones((1, HD0), np.float32))).astype(ml_dtypes.bfloat16),
        })
    return in_maps


_NC_CACHE = {}


def get_nc():
    if "nc" not in _NC_CACHE:
        _NC_CACHE["nc"] = build_nc()
    return _NC_CACHE["nc"]


def run(inputs, trace=False, **kwargs):
    """Run the SPMD kernel; returns (full_output, BassKernelResults)."""
    x = np.ascontiguousarray(np.asarray(inputs["x"], dtype=np.float32))
    Wqkv = np.asarray(inputs["Wqkv"], dtype=np.float32)
    bqkv = np.asarray(inputs["bqkv"], dtype=np.float32)
    Wout = np.asarray(inputs["Wout"], dtype=np.float32)
    bout = np.asarray(inputs["bout"], dtype=np.float32)
    r, E3, D = Wqkv.shape
    Bb, S, _ = x.shape
    DC = D // P

    nc = get_nc()
    in_maps = make_in_maps(x, Wqkv, bqkv, Wout, bout)
    res = run_bass_kernel_spmd(nc, in_maps, core_ids=list(range(len(in_maps))),
                               trace=trace, **kwargs)

    out = np.zeros((Bb, S, r * D), np.float32)
    for c in range(len(in_maps)):
        o, b = c // Bb, c % Bb
        # unpack fc-major [NSEG, DC, P, SEG] -> [L, D] (tokens, channels)
        op = res.results[c]["out_pk"].reshape(NSEG0, DC, P, SEG0)
        oT = op.transpose(0, 3, 1, 2).reshape(S // r, D)
        out[b, o::r, o * D:(o + 1) * D] = oT
    return out, res


def kernel(x, Wqkv, bqkv, Wout, bout, num_heads):
    assert int(num_heads) == H0
    out, _ = run(dict(x=x, Wqkv=Wqkv, bqkv=bqkv, Wout=Wout, bout=bout))
    return out


# revision 55
# speedup vs baseline: 1.1118x; 1.1118x over previous
"""Blocksparse dilated attention TRN2 kernel.

Sharding: 8 cores = r(=4 dilation offsets) x B(=2 batch). Each core runs one
independent per-offset attention branch on its strided token subset
(x[b, o::r, :]), with that offset's own weights. Host does the strided
gather (+pack to the kernel's DMA-friendly layouts) and the final scatter
into the zero-padded (B, S, r*D) output.

Per-core math (L=2048 tokens, D=768, H=12 heads, hd=64, segment=512):
  qkvT = Wqkv @ xoT            (channel-on-partition for q,k; token-major v)
  per (segment, head):  scoresT = kT-chunks.T x qT   (k on partitions)
                        attnT = exp(scale * scoresT)  (no max-subtract:
                              scores std ~0.3, max ~1.5 -> exp safe in fp32)
                        [ctxu; denom] = [v | ones].T @ attnT  (ones column
                              makes PSUM row 64 the softmax denominator)
  per (segment, chunk): rc = 1/denom  (DVE reciprocal after a DMA spread
                              across 128 partitions)
                        ctxT = ctxu * broadcast(rc)  (rc staged to DRAM,
                              then partition-step-0 broadcast DMA per chunk)
  outT = Wout @ ctxT + bout

Matmuls run in bf16 (full PE rate; fp32 PSUM accumulation). fp32/fp32r
matmuls are unusable here (walrus single sync-wait slot on fused
load+matmul); fp8 was measured on CPU at rel_err 1.9e-2 even in the
safest (q,k-proj only) variant -- too close to the 2e-2 gate.

Scheduling notes (each earned from a perfetto trace):
- Emission order software-pipelines segments so the PE never idles long
  enough (~3.4us) for the HAM clock gate to drop it from 2.4 to 1.2 GHz:
  proj(s) -> [normalize(s-1) + outproj(s-1)] -> attention(s).
- The reciprocal chain (den-copy -> spread-DMA -> DVE recip -> writeback)
  is latency-bound (~2us per DMA hop). Its stages are emitted STAGGERED
  one chunk apart so every queue item's wait is ~zero when it reaches its
  engine's head: an in-order engine queue head-of-line-blocks on any long
  wait (this serialized the whole tail when all stages sat adjacent on
  the gpsimd queue).
- For segments whose normalize runs a full segment later, the reciprocal
  rows go to DRAM and come back via a gpsimd SWDGE partition-step-0
  broadcast DMA (the only engine allowed such a source AP). For the FINAL
  segment that path is unusable: SWDGE broadcast completion latency was
  measured at 5-16us. Instead the spread reciprocal is linearized by one
  small SBUF->SBUF DMA into [2, SEG] rows and broadcast across the 128
  partitions by a K=2 PE matmul against a host-provided block-selection
  mask (sel2), with the ensuing multiply reading the PSUM product.
- The final tail keeps the PE busy (and HAM warm) while the last chunks'
  chains drain: segment SL-1's outproj columns 4,5 are withheld from the
  filler budget and emitted in the tail, plus a 2-bank dc-progressive
  accumulation for SL's first two outproj columns.
- Weights/xo are DMA'd from HOST-PACKED buffers: each descriptor reads
  DRAM fully sequentially (a strided [p, dc, e] gather out of the natural
  [D, E3] layout measured ~53 GB/s; packed runs near peak, but a single
  HWDGE queue still only sustains ~90 GB/s). The qk weight stream issues
  on sync and xo/v/bias bytes on scalar so the two streams transfer in
  parallel during the cold start; output stores also ride scalar so they
  never head-of-line block the tail's small latency-critical sync DMAs.
"""

import math
import sys
from contextlib import ExitStack

import ml_dtypes
import numpy as np

for _p in ("/opt/trn_rl_repo",):
    if _p not in sys.path:
        sys.path.insert(0, _p)

import concourse.bass as bass
import concourse.mybir as mybir
import concourse.tile as tile
from concourse import bacc
from concourse.bass_utils import run_bass_kernel_spmd

P = 128

# Problem constants (hardcoded per harness contract)
B0, S0, D0 = 2, 8192, 768
R0 = 4
H0, HD0 = 12, 64
SEG0 = 512
NSEG0 = (S0 // R0) // SEG0  # 4
N_CORES = 8

F32 = mybir.dt.float32
BF16 = mybir.dt.bfloat16

# qk weight stream descriptor groups (dc0, ndc, col0, ncols within the 2*D
# qk sections): the first groups gate the very first matmul groups so they
# get small dc-split descriptors; the rest come in column pairs.
DC0 = D0 // P
QK_GROUPS = ([(0, 3, 0, P), (3, 3, 0, P), (0, DC0, P, P)]
             + [(0, DC0, c, 2 * P) for c in range(2 * P, 2 * D0, 2 * P)])
V_GROUPS = [(0, DC0, 0, 512), (0, DC0, 512, D0 - 512)]  # cols within v section
# segment-0 xo split: (dc0) tiny first on scalar to unblock the first MM,
# (dc1-2) on scalar, (dc3-5) interleaved into the sync weight stream
XO0_GROUPS = [(0, 1), (1, 2), (3, 3)]


def build_nc(D=D0, H=H0, HD=HD0, SEG=SEG0, NSEG=NSEG0, mm_dt=BF16):
    """Build the per-core Bass program (same NEFF on all cores)."""
    DC = D // P                # channel chunks of 128
    L = SEG * NSEG             # tokens per core
    KC = SEG // P              # key chunks per segment
    HPC = P // HD              # heads per 128-channel chunk
    E3 = 3 * D
    HV = HD + 1                # v columns per head incl. ones column
    scale = 1.0 / math.sqrt(HD)
    assert D == H * HD and SEG % P == 0 and D % P == 0 and KC % 2 == 0

    nc = bacc.Bacc(trn_type="TRN2")
    xo_pk = nc.dram_tensor("xo_pk", [NSEG, D * SEG], mm_dt, kind="ExternalInput")
    wqk_pk = nc.dram_tensor("wqk_pk", [2 * D * D], mm_dt, kind="ExternalInput")
    wv_pk = nc.dram_tensor("wv_pk", [D * D], mm_dt, kind="ExternalInput")
    wout_pk = nc.dram_tensor("wout_pk", [D * D], mm_dt, kind="ExternalInput")
    bqkv_pt = nc.dram_tensor("bqkv_pt", [P, 3 * DC], F32, kind="ExternalInput")
    bout_pt = nc.dram_tensor("bout_pt", [P, DC], F32, kind="ExternalInput")
    bv = nc.dram_tensor("bv", [D], F32, kind="ExternalInput")
    sel2_d = nc.dram_tensor("sel2_d", [P // HD, P], mm_dt, kind="ExternalInput")
    out_pk = nc.dram_tensor("out_pk", [NSEG, D * SEG], F32, kind="ExternalOutput")
    # scratch for the softmax reciprocals: broadcast-DMA needs a DRAM source
    # (SBUF-source partition-step-0 APs are rejected)
    rc_dram = nc.dram_tensor("rc_dram", [NSEG, H * SEG], mm_dt, kind="Internal")

    def packed_ap(t, flat_off, ncols, ndc=None):
        """AP over a host-packed [P, ndc, ncols] block starting at flat_off.
        The element stream is fully sequential in DRAM."""
        if ndc is None:
            ndc = DC
        base = t[:]
        return bass.AP(tensor=base.tensor, offset=flat_off,
                       ap=[[ndc * ncols, P], [ncols, ndc], [1, ncols]])

    with ExitStack() as ctx:
        tc = ctx.enter_context(tile.TileContext(nc))
        singles = ctx.enter_context(tc.tile_pool(name="singles", bufs=1))
        xo_pool = ctx.enter_context(tc.tile_pool(name="xo", bufs=2))
        qk_pool = ctx.enter_context(tc.tile_pool(name="qk", bufs=2))
        v_pool = ctx.enter_context(tc.tile_pool(name="v", bufs=2))
        attn_pool = ctx.enter_context(tc.tile_pool(name="attn", bufs=3))
        ctxu_pool = ctx.enter_context(tc.tile_pool(name="ctxu", bufs=2))
        den_pool = ctx.enter_context(tc.tile_pool(name="den", bufs=1))
        rc_pool = ctx.enter_context(tc.tile_pool(name="rc", bufs=2))
        rows_pool = ctx.enter_context(tc.tile_pool(name="rcrows", bufs=6))
        ctxs_pool = ctx.enter_context(tc.tile_pool(name="ctxs", bufs=2))
        out_pool = ctx.enter_context(tc.tile_pool(name="outp", bufs=2))
        bcast_pool = ctx.enter_context(tc.tile_pool(name="bcast", bufs=6))
        pp_proj = ctx.enter_context(tc.tile_pool(name="pp_proj", bufs=2, space="PSUM"))
        pp_scA = ctx.enter_context(tc.tile_pool(name="pp_scA", bufs=1, space="PSUM"))
        pp_scB = ctx.enter_context(tc.tile_pool(name="pp_scB", bufs=1, space="PSUM"))
        pp_cb = ctx.enter_context(tc.tile_pool(name="pp_cb", bufs=2, space="PSUM"))

        # --- segment-0 xo halves, one per HWDGE queue (a single queue only
        # sustains ~90 GB/s, so the two descriptor streams transfer the
        # cold-start-critical bytes in parallel) ---
        xo_first = xo_pool.tile([P, DC, SEG], mm_dt, tag="xo", name="xo_s0")
        xo0_off = {}
        off = 0
        for dc0, ndc in XO0_GROUPS:
            xo0_off[dc0] = (off, ndc)
            off += P * ndc * SEG

        def xo0_dma(eng, dc0):
            o, ndc = xo0_off[dc0]
            eng.dma_start(out=xo_first[:, dc0:dc0 + ndc, :],
                          in_=packed_ap(xo_pk, o, SEG, ndc))

        xo0_dma(nc.scalar, 0)        # 128 KB: unblocks the very first MM
        xo0_dma(nc.scalar, 1)

        # tiny biases (a late bias DMA stalls the first qk-add on DVE and
        # cascades into a PE psum-WAR stall)
        bqkv_sb = singles.tile([P, 3 * DC], F32, tag="bqkv")
        nc.scalar.dma_start(out=bqkv_sb, in_=bqkv_pt[:, :])
        bout_sb = singles.tile([P, DC], F32, tag="bout")
        nc.scalar.dma_start(out=bout_sb, in_=bout_pt[:, :])
        # v-section bias broadcast along partitions (natural layout add)
        bv_sb = singles.tile([P, D], F32, tag="bv")
        bv_ap = bv[:]
        bv_bcast = bass.AP(tensor=bv_ap.tensor, offset=bv_ap.offset,
                           ap=[[0, P], *bv_ap.ap])
        nc.gpsimd.dma_start(out=bv_sb, in_=bv_bcast)

        # qk weight stream on sync, in first-use (column-group) order, with
        # the second xo0 half interleaved after the first weight group
        w_qkv_sb = singles.tile([P, DC, E3], mm_dt, tag="wqkv")
        off = 0
        for gi, (dc0, ndc, c0, nco) in enumerate(QK_GROUPS):
            nc.sync.dma_start(out=w_qkv_sb[:, dc0:dc0 + ndc, c0:c0 + nco],
                              in_=packed_ap(wqk_pk, off, nco, ndc))
            off += P * ndc * nco
            if gi == 1:
                xo0_dma(nc.sync, 3)
        off = 0
        for dc0, ndc, c0, nco in V_GROUPS:
            nc.scalar.dma_start(out=w_qkv_sb[:, dc0:dc0 + ndc,
                                             2 * D + c0:2 * D + c0 + nco],
                                in_=packed_ap(wv_pk, off, nco, ndc))
            off += P * ndc * nco
        # wout on the (otherwise idle) gpsimd queue: it is needed last and
        # this keeps the scalar queue free for xo/v-weight bytes
        w_out_sb = singles.tile([P, DC, D], mm_dt, tag="wout")
        nc.gpsimd.dma_start(out=w_out_sb[:, :, :], in_=packed_ap(wout_pk, 0, D))

        # 2x128 block-selection mask for the tail's PE-broadcast of the
        # reciprocal rows (sel2[h, m] = 1 iff m//HD == h); host-provided
        # (single-partition memsets at partition base 1 fail BIR verification)
        sel2 = singles.tile([HPC, P], mm_dt, tag="sel2")
        nc.scalar.dma_start(out=sel2, in_=sel2_d[:, :])
        # scratch sink for the cold-start warm-up matmuls (keeps the BIR
        # verifier happy: every written location needs a reader)
        dmy_rd = singles.tile([1, 16], F32, tag="dmyrd")

        def load_and_proj(s, pre_tasks=()):
            """xo load + qkv projections for segment s. pre_tasks are emitted
            after the first two qk chunks (prev segment's last recip chain --
            by then its dent DMA has landed)."""
            st = {}
            if s == 0:
                xo_s = xo_first
            else:
                xo_s = xo_pool.tile([P, DC, SEG], mm_dt, tag="xo", name=f"xo_s{s}")
                nc.scalar.dma_start(out=xo_s[:, :, :],
                                    in_=packed_ap(xo_pk, s * D * SEG, SEG))
            st["xo"] = xo_s
            st["ctxu"] = ctxu_pool.tile([P, DC, SEG], mm_dt, tag="ctxu",
                                        name=f"ctxu{s}")
            st["den"] = den_pool.tile([1, H * SEG], F32, tag="den",
                                      name=f"den{s}")
            st["ctx_s"] = ctxs_pool.tile([P, DC, SEG], mm_dt, tag="ctxs",
                                         name=f"cs{s}")
            st["out_seg"] = out_pool.tile([P, DC, SEG], F32, tag="ot",
                                          name=f"ot{s}")

            # q,k in transposed layout (e on partitions)
            qk_s = qk_pool.tile([P, 2 * DC, SEG], mm_dt, tag="qk", name=f"qk_s{s}")
            st["qk"] = qk_s
            for ec in range(2 * DC):
                ps = pp_proj.tile([P, SEG], F32, tag="proj", name=f"psqk{s}_{ec}")
                for dc in range(DC):
                    nc.tensor.matmul(
                        ps,
                        w_qkv_sb[:, dc, ec * P:(ec + 1) * P],
                        xo_s[:, dc, :],
                        start=(dc == 0), stop=(dc == DC - 1))
                nc.vector.tensor_scalar_add(qk_s[:, ec, :], ps, bqkv_sb[:, ec:ec + 1])
                if s == 0 and 1 <= ec <= 5:
                    # cold-start warm-up: the DMA supply can only feed the
                    # PE an ec-group every ~2.5us here; these throwaway K=2
                    # matmuls keep the PE busy through the supply gaps so
                    # the HAM clock gate reaches 2.4 GHz ~10us sooner
                    dmy = pp_proj.tile([P, SEG], F32, tag="proj",
                                       name=f"dmy{ec}")
                    for r_ in range(5):
                        nc.tensor.matmul(dmy, xo_s[0:2, 0, 0:P],
                                         xo_s[0:2, 0, :],
                                         start=(r_ == 0), stop=(r_ == 4))
                    nc.vector.tensor_copy(dmy_rd, dmy[0:1, 0:16])
                if ec == 1:
                    for t in pre_tasks:
                        t()

            # v in natural layout (token on partitions), per-head + ones column
            v_s = v_pool.tile([P, KC, H * HV], mm_dt, tag="v", name=f"v_s{s}")
            st["v"] = v_s
            v_view = v_s.rearrange("p k (h c) -> p k h c", c=HV)
            nc.vector.memset(v_view[:, :, :, HD:HD + 1], 1.0)
            for lc in range(KC):
                for n0 in range(0, D, 512):
                    n = min(512, D - n0)
                    nh = n // HD
                    h0 = n0 // HD
                    psv = pp_proj.tile([P, SEG], F32, tag="proj",
                                       name=f"psv{s}_{lc}_{n0}")
                    for dc in range(DC):
                        nc.tensor.matmul(
                            psv[:, :n],
                            xo_s[:, dc, lc * P:(lc + 1) * P],
                            w_qkv_sb[:, dc, 2 * D + n0: 2 * D + n0 + n],
                            start=(dc == 0), stop=(dc == DC - 1))
                    nc.vector.tensor_add(
                        v_view[:, lc, h0:h0 + nh, 0:HD],
                        psv[:, :n].rearrange("p (h c) -> p h c", c=HD),
                        bv_sb[:, n0:n0 + n].rearrange("p (h c) -> p h c", c=HD))
            return st

        def dent_dma(s, st, hc0, nhc, tag):
            """Spread den elements for chunks [hc0,hc0+nhc) over 128
            partitions (DVE reciprocal is ~8 cyc/elem/lane; a 1-partition
            strip would take ~6us; element order is irrelevant since the
            writeback DMA restores it)."""
            den = st["den"]
            e0, ne = hc0 * HPC * SEG, nhc * HPC * SEG
            den_t = rc_pool.tile([P, ne // P], F32, tag=f"dent{tag}",
                                 name=f"dent{s}_{hc0}")
            nc.sync.dma_start(out=den_t, in_=den[0:1, e0:e0 + ne])
            st[f"dent_{hc0}_{nhc}"] = den_t

        def recip_writeback(s, st, hc0, nhc, tag, to_rows=False):
            """DVE reciprocal of the spread denominators, then either write
            back to DRAM (for the SWDGE broadcast path) or, for the tail
            chunks, a single SBUF->SBUF linearizing DMA into a [HPC, SEG]
            row tile consumed by the PE-broadcast (the SWDGE broadcast has
            ~5-7us completion latency that the tail cannot hide)."""
            den_t = st.pop(f"dent_{hc0}_{nhc}")
            e0, ne = hc0 * HPC * SEG, nhc * HPC * SEG
            rc_t = rc_pool.tile([P, ne // P], mm_dt, tag=f"rct{tag}",
                                name=f"rct{s}_{hc0}")
            with nc.allow_low_precision(
                    reason="softmax denominator reciprocal; bf16 scale factor"):
                nc.vector.reciprocal(rc_t, den_t)
            if to_rows:
                rows = rows_pool.tile([HPC, SEG], mm_dt, tag="rcrows",
                                      name=f"rcrows{s}_{hc0}")
                nc.sync.dma_start(out=rows, in_=rc_t[:, :])
                st.setdefault("rcrows", {})[hc0] = rows
            else:
                nc.sync.dma_start(out=rc_dram[s:s + 1, e0:e0 + ne], in_=rc_t)

        def attention(s, st, filler=(), per_chunk_recip=False):
            """scores + exp + unnormalized ctx (and denom), processed in
            head PAIRS: the two heads of a 128-channel chunk occupy PE
            row-groups 0-63 and 64-127, and their K=64 scores matmuls are
            emitted adjacently so the array runs them concurrently (~2x on
            the scores phase). Pipelined: ctx(pair-1) after scores(pair).
            `filler` tasks (prev segment's normalize + outproj) are emitted
            between pairs so the PE has work while ACT exp catches up.
            Returns deferred end-of-segment recip tasks for the caller to
            emit inside the next segment's projection."""
            filler = list(filler)
            n_filler = len(filler)
            emitted = 0
            qk_s, v_s = st["qk"], st["v"]
            ctxu, den = st["ctxu"], st["den"]
            ats = {}
            HH = DC // 2
            for c in range(DC + 1):
                while emitted < (c * n_filler) // DC:
                    filler[emitted]()
                    emitted += 1
                if c < DC:
                    at2 = attn_pool.tile([P, HPC, KC, SEG], mm_dt, tag="attn",
                                         name=f"at{s}_{c}")
                    ats[c] = at2
                    for w in range(KC // 2):
                        for half, pool in ((0, pp_scA), (1, pp_scB)):
                            kc = 2 * w + half
                            sc = pool.tile([P, HPC, SEG], F32, tag=f"sc{half}",
                                           name=f"sc{half}_{s}_{c}_{w}")
                            for i in range(HPC):
                                ho = i * HD
                                nc.tensor.matmul(
                                    sc[:, i, :],
                                    qk_s[ho:ho + HD, DC + c, kc * P:(kc + 1) * P],
                                    qk_s[ho:ho + HD, c, :])
                            nc.scalar.activation(
                                at2[:, :, kc, :], sc,
                                mybir.ActivationFunctionType.Exp,
                                scale=scale)
                if c > 0:
                    cp = c - 1
                    at2 = ats.pop(cp)
                    for i in range(HPC):
                        h = cp * HPC + i
                        ho = i * HD
                        cps = pp_cb.tile([P, SEG], F32, tag="cb",
                                         name=f"cps{s}_{h}")
                        for kc in range(KC):
                            nc.tensor.matmul(
                                cps[0:HD + 1, :],
                                v_s[:, kc, h * HV:(h + 1) * HV],
                                at2[:, i, kc, :],
                                start=(kc == 0), stop=(kc == KC - 1))
                        # den copy first: it is on the reciprocal-chain
                        # critical path, the ctxu cast is not
                        nc.vector.tensor_copy(den[0:1, h * SEG:(h + 1) * SEG],
                                              cps[HD:HD + 1, :])
                        nc.vector.tensor_copy(ctxu[ho:ho + HD, cp, :],
                                              cps[0:HD, :])
                    # staggered reciprocal chains: each stage is emitted a
                    # chunk after its producer so no engine-queue head ever
                    # waits long (HOL blocking)
                    if per_chunk_recip:
                        # all chunks of the final segment use the PE-broadcast
                        # rows path: SWDGE broadcast completion latency was
                        # measured at 5-16us and stalls the tail's DVE queue
                        dent_dma(s, st, cp, 1, "c")
                        if cp >= 1:
                            recip_writeback(s, st, cp - 1, 1, "c",
                                            to_rows=True)
                    else:
                        if cp == HH - 1:
                            dent_dma(s, st, 0, HH, "h")
                        elif cp == HH + 1:
                            recip_writeback(s, st, 0, HH, "h")
                            # pre-issue the SWDGE broadcasts now: their
                            # 5-16us completion latency must be paid long
                            # before the next segment's normalize fillers
                            for hc in range(HH):
                                bcast_chunk(s, st, hc)
                        elif cp == DC - 1:
                            dent_dma(s, st, HH, DC - HH, "h")
            if per_chunk_recip:
                return [lambda: recip_writeback(s, st, DC - 1, 1, "c",
                                                to_rows=True)]

            def second_half():
                recip_writeback(s, st, HH, DC - HH, "h")
                for hc in range(HH, DC):
                    bcast_chunk(s, st, hc)
            return [second_half]

        def bcast_chunk(s, st, hc):
            """Broadcast the 2 per-head reciprocal rows of chunk hc across HD
            partitions by an SWDGE DMA (partition-step-0 source AP) instead
            of a PE outer-product."""
            bcs = bcast_pool.tile([P, SEG], mm_dt, tag="bcs",
                                  name=f"bcs{s}_{hc}")
            rr = rc_dram[s:s + 1, hc * HPC * SEG:(hc + 1) * HPC * SEG]
            rr_b = bass.AP(tensor=rr.tensor, offset=rr.offset,
                           ap=[[SEG, HPC], [0, HD], [1, SEG]])
            nc.gpsimd.dma_start(out=bcs, in_=rr_b)
            st.setdefault("bcs", {})[hc] = bcs
            return bcs

        def norm_chunk(s, st, hc):
            """Normalize one 128-channel (2-head) chunk."""
            ctxu, ctx_s = st["ctxu"], st["ctx_s"]
            rows = st.get("rcrows", {}).pop(hc, None)
            if rows is not None:
                # tail path: PE outer-product broadcast (K=2 matmul against
                # the block-selection mask) instead of the slow SWDGE DMA
                bc = pp_cb.tile([P, SEG], F32, tag="cb", name=f"bcps{s}_{hc}")
                nc.tensor.matmul(bc, sel2, rows)
                nc.vector.tensor_mul(ctx_s[:, hc, :], ctxu[:, hc, :], bc)
                return
            bcs = st.get("bcs", {}).pop(hc, None)
            if bcs is None:
                bcs = bcast_chunk(s, st, hc)
                st["bcs"].pop(hc)
            nc.vector.tensor_mul(ctx_s[:, hc, :], ctxu[:, hc, :], bcs)

        def outproj_fc(s, st, fc, pool=None, tag="proj"):
            """Full out-projection column fc (all DC contraction chunks)."""
            ctx_s, out_seg = st["ctx_s"], st["out_seg"]
            pso = (pool or pp_proj).tile([P, SEG], F32, tag=tag,
                                         name=f"pso{s}_{fc}")
            for dc in range(DC):
                nc.tensor.matmul(
                    pso,
                    w_out_sb[:, dc, fc * P:(fc + 1) * P],
                    ctx_s[:, dc, :],
                    start=(dc == 0), stop=(dc == DC - 1))
            nc.vector.tensor_scalar_add(out_seg[:, fc, :], pso,
                                        bout_sb[:, fc:fc + 1])

        def store_out(s, st):
            # out_pk is fc-major [NSEG, DC, P, SEG]; this AP enumerates it
            # in the tile's (p, dc, t) order. Stores ride the scalar queue:
            # a 1.5MB store on sync would head-of-line block the tail's
            # small latency-critical DMAs.
            base = out_pk[:]
            nc.scalar.dma_start(
                out=bass.AP(tensor=base.tensor, offset=s * D * SEG,
                            ap=[[SEG, P], [P * SEG, DC], [1, SEG]]),
                in_=st["out_seg"][:, :, :])

        def store_fc(s, st, fc, eng):
            """Store one out-projection column as soon as its bias-add is
            done (the final segment's single 1.5MB store exposed ~5.5us of
            pure drain after the last matmul)."""
            base = out_pk[:]
            eng.dma_start(
                out=bass.AP(tensor=base.tensor,
                            offset=(s * DC + fc) * P * SEG,
                            ap=[[SEG, P], [1, SEG]]),
                in_=st["out_seg"][:, fc, :])

        def norm_tasks(s, st, nfc=DC, store=True):
            """Deferred normalize + outproj tasks (run as PE/DVE filler inside
            the next segment's attention). nfc/store limit how much outproj
            is spent as filler -- the rest is saved as tail fill."""
            tasks = ([(lambda hc=hc: norm_chunk(s, st, hc)) for hc in range(DC)]
                     + [(lambda fc=fc: outproj_fc(s, st, fc)) for fc in range(nfc)])
            if store:
                tasks.append(lambda: store_out(s, st))
            return tasks

        sts = {}
        SL = NSEG - 1
        deferred = []
        for s in range(NSEG):
            sts[s] = load_and_proj(s, pre_tasks=deferred)
            if s > 0:
                filler = list(norm_tasks(s - 1, sts[s - 1], nfc=4 if s == SL else DC,
                                         store=(s != SL)))
                if s != SL:
                    sts.pop(s - 1)
            else:
                filler = []
            deferred = attention(s, sts[s], filler, per_chunk_recip=(s == SL))

        # --- final segment tail. The reciprocal chains of the last two
        # chunks have ~8us of unavoidable DMA latency; the saved outproj
        # work of segment SL-1 (fc 4,5) plus a 2-bank dc-progressive for
        # SL's fc 0,1 keeps the PE busy (and HAM warm) while they drain;
        # fc 2-5 of SL run as ordinary full groups at the end. ---
        stp, stl = sts.pop(SL - 1), sts.pop(SL)
        ctx_s, out_seg = stl["ctx_s"], stl["out_seg"]
        norm_chunk(SL, stl, 0)
        norm_chunk(SL, stl, 1)
        for t in deferred:
            t()
        outproj_fc(SL - 1, stp, 4)
        outproj_fc(SL - 1, stp, 5)
        store_out(SL - 1, stp)
        n_prog = 2
        psos = {}
        for fc in range(n_prog):
            psos[fc] = pp_proj.tile([P, SEG], F32, tag="proj", name=f"psoL_{fc}")
        for dc in range(DC):
            if dc >= 2:
                norm_chunk(SL, stl, dc)
            for fc in range(n_prog):
                nc.tensor.matmul(
                    psos[fc],
                    w_out_sb[:, dc, fc * P:(fc + 1) * P],
                    ctx_s[:, dc, :],
                    start=(dc == 0), stop=(dc == DC - 1))
        for fc in range(n_prog):
            nc.vector.tensor_scalar_add(out_seg[:, fc, :], psos[fc],
                                        bout_sb[:, fc:fc + 1])
            store_fc(SL, stl, fc, nc.scalar if fc % 2 else nc.sync)
        for fc in range(n_prog, DC):
            # alternate PSUM pools: a 2-bank rotation stalls each trailing
            # group ~1.3us on the previous group's TS drain (WAR)
            outproj_fc(SL, stl, fc,
                       pool=(pp_cb if fc % 2 else pp_proj),
                       tag=("cb" if fc % 2 else "proj"))
            store_fc(SL, stl, fc, nc.scalar if fc % 2 else nc.sync)

    nc.compile()
    return nc


def _pack_w(wT, c0, ncols, dc0=0, ndc=D0 // P):
    """[D, cols] slice -> packed [P, ndc, ncols] stream (fully sequential)."""
    return np.ascontiguousarray(
        wT[:, c0:c0 + ncols].reshape(D0 // P, P, ncols)[dc0:dc0 + ndc]
        .transpose(1, 0, 2)).ravel()


def make_in_maps(x, Wqkv, bqkv, Wout, bout):
    """Shard full inputs across 8 cores: core = o*B + b."""
    r, E3, D = Wqkv.shape
    Bb, S, _ = x.shape
    DC = D // P
    in_maps = []
    for c in range(r * Bb):
        o, b = c // Bb, c % Bb
        wT = np.ascontiguousarray(Wqkv[o].T).astype(ml_dtypes.bfloat16)
        woT = np.ascontiguousarray(Wout[o].T).astype(ml_dtypes.bfloat16)
        xoT = np.ascontiguousarray(x[b, o::r, :].T).astype(ml_dtypes.bfloat16)
        wqk_pk = np.concatenate([_pack_w(wT, c0, nco, dc0, ndc)
                                 for dc0, ndc, c0, nco in QK_GROUPS])
        wv_pk = np.concatenate([_pack_w(wT, 2 * D + c0, nco, dc0, ndc)
                                for dc0, ndc, c0, nco in V_GROUPS])
        wout_pk = _pack_w(woT, 0, D)
        xo0 = np.concatenate([_pack_w(xoT, 0, SEG0, dc0, ndc)
                              for dc0, ndc in XO0_GROUPS])
        xo_pk = np.stack([xo0] + [_pack_w(xoT, s * SEG0, SEG0)
                                  for s in range(1, NSEG0)])
        in_maps.append({
            "xo_pk": xo_pk,
            "wqk_pk": wqk_pk,
            "wv_pk": wv_pk,
            "wout_pk": wout_pk,
            "bqkv_pt": np.ascontiguousarray(bqkv[o].reshape(3 * DC, P).T),
            "bout_pt": np.ascontiguousarray(bout[o].reshape(DC, P).T),
            "bv": np.ascontiguousarray(bqkv[o, 2 * D:3 * D]),
            "sel2_d": np.ascontiguousarray(
                np.kron(np.eye(P // HD0, dtype=np.float32),
                        np.ones((1, HD0), np.float32))).astype(ml_dtypes.bfloat16),
        })
    return in_maps


_NC_CACHE = {}


def get_nc():
    if "nc" not in _NC_CACHE:
        _NC_CACHE["nc"] = build_nc()
    return _NC_CACHE["nc"]


def run(inputs, trace=False, **kwargs):
    """Run the SPMD kernel; returns (full_output, BassKernelResults)."""
    x = np.ascontiguousarray(np.asarray(inputs["x"], dtype=np.float32))
    Wqkv = np.asarray(inputs["Wqkv"], dtype=np.float32)
    bqkv = np.asarray(inputs["bqkv"], dtype=np.float32)
    Wout = np.asarray(inputs["Wout"], dtype=np.float32)
    bout = np.asarray(inputs["bout"], dtype=np.float32)
    r, E3, D = Wqkv.shape
    Bb, S, _ = x.shape
    DC = D // P

    nc = get_nc()
    in_maps = make_in_maps(x, Wqkv, bqkv, Wout, bout)
    res = run_bass_kernel_spmd(nc, in_maps, core_ids=list(range(len(in_maps))),
                               trace=trace, **kwargs)

    out = np.zeros((Bb, S, r * D), np.float32)
    for c in range(len(in_maps)):
        o, b = c // Bb, c % Bb
        # unpack fc-major [NSEG, DC, P, SEG] -> [L, D] (tokens, channels)
        op = res.results[c]["out_pk"].reshape(NSEG0, DC, P, SEG0)
        oT = op.transpose(0, 3, 1, 2).reshape(S // r, D)
        out[b, o::r, o * D:(o + 1) * D] = oT
    return out, res


def kernel(x, Wqkv, bqkv, Wout, bout, num_heads):
    assert int(num_heads) == H0
    out, _ = run(dict(x=x, Wqkv=Wqkv, bqkv=bqkv, Wout=Wout, bout=bout))
    return out


# revision 56
# speedup vs baseline: 1.1608x; 1.0441x over previous
"""Blocksparse dilated attention TRN2 kernel.

Sharding: 8 cores = r(=4 dilation offsets) x B(=2 batch). Each core runs one
independent per-offset attention branch on its strided token subset
(x[b, o::r, :]), with that offset's own weights. Host does the strided
gather (+pack to the kernel's DMA-friendly layouts) and the final scatter
into the zero-padded (B, S, r*D) output.

Per-core math (L=2048 tokens, D=768, H=12 heads, hd=64, segment=512):
  qkvT = Wqkv @ xoT            (channel-on-partition for q,k; token-major v)
  per (segment, head):  scoresT = kT-chunks.T x qT   (k on partitions)
                        attnT = exp(scale * scoresT)  (no max-subtract:
                              scores std ~0.3, max ~1.5 -> exp safe in fp32)
                        [ctxu; denom] = [v | ones].T @ attnT  (ones column
                              makes PSUM row 64 the softmax denominator)
  per (segment, chunk): rc = 1/denom  (DVE reciprocal after a DMA spread
                              across 128 partitions)
                        ctxT = ctxu * broadcast(rc)  (rc staged to DRAM,
                              then partition-step-0 broadcast DMA per chunk)
  outT = Wout @ ctxT + bout

Matmuls run in bf16 (full PE rate; fp32 PSUM accumulation). fp32/fp32r
matmuls are unusable here (walrus single sync-wait slot on fused
load+matmul); fp8 was measured on CPU at rel_err 1.9e-2 even in the
safest (q,k-proj only) variant -- too close to the 2e-2 gate.

Scheduling notes (each earned from a perfetto trace):
- Emission order software-pipelines segments so the PE never idles long
  enough (~3.4us) for the HAM clock gate to drop it from 2.4 to 1.2 GHz:
  proj(s) -> [normalize(s-1) + outproj(s-1)] -> attention(s).
- The reciprocal chain (den-copy -> spread-DMA -> DVE recip -> writeback)
  is latency-bound (~2us per DMA hop). Its stages are emitted STAGGERED
  one chunk apart so every queue item's wait is ~zero when it reaches its
  engine's head: an in-order engine queue head-of-line-blocks on any long
  wait (this serialized the whole tail when all stages sat adjacent on
  the gpsimd queue).
- For segments whose normalize runs a full segment later, the reciprocal
  rows go to DRAM and come back via a gpsimd SWDGE partition-step-0
  broadcast DMA (the only engine allowed such a source AP). For the FINAL
  segment that path is unusable: SWDGE broadcast completion latency was
  measured at 5-16us. Instead the spread reciprocal is linearized by one
  small SBUF->SBUF DMA into [2, SEG] rows and broadcast across the 128
  partitions by a K=2 PE matmul against a host-provided block-selection
  mask (sel2), with the ensuing multiply reading the PSUM product.
- The final tail keeps the PE busy (and HAM warm) while the last chunks'
  chains drain: segment SL-1's outproj columns 4,5 are withheld from the
  filler budget and emitted in the tail, plus a 2-bank dc-progressive
  accumulation for SL's first two outproj columns.
- Weights/xo are DMA'd from HOST-PACKED buffers: each descriptor reads
  DRAM fully sequentially (a strided [p, dc, e] gather out of the natural
  [D, E3] layout measured ~53 GB/s; packed runs near peak, but a single
  HWDGE queue still only sustains ~90 GB/s). The qk weight stream issues
  on sync and xo/v/bias bytes on scalar so the two streams transfer in
  parallel during the cold start; output stores also ride scalar so they
  never head-of-line block the tail's small latency-critical sync DMAs.
"""

import math
import sys
from contextlib import ExitStack

import ml_dtypes
import numpy as np

for _p in ("/opt/trn_rl_repo",):
    if _p not in sys.path:
        sys.path.insert(0, _p)

import concourse.bass as bass
import concourse.mybir as mybir
import concourse.tile as tile
from concourse import bacc
from concourse.bass_utils import run_bass_kernel_spmd

P = 128

# Problem constants (hardcoded per harness contract)
B0, S0, D0 = 2, 8192, 768
R0 = 4
H0, HD0 = 12, 64
SEG0 = 512
NSEG0 = (S0 // R0) // SEG0  # 4
N_CORES = 8

F32 = mybir.dt.float32
BF16 = mybir.dt.bfloat16

# qk weight stream descriptor groups (dc0, ndc, col0, ncols within the 2*D
# qk sections): the first groups gate the very first matmul groups so they
# get small dc-split descriptors; the rest come in column pairs.
DC0 = D0 // P
QK_GROUPS = ([(0, 3, 0, P), (3, 3, 0, P), (0, DC0, P, P)]
             + [(0, DC0, c, 2 * P) for c in range(2 * P, 2 * D0, 2 * P)])
V_GROUPS = [(0, DC0, 0, 512), (0, DC0, 512, D0 - 512)]  # cols within v section
# segment-0 xo split: (dc0) tiny first on scalar to unblock the first MM,
# (dc1-2) on scalar, (dc3-5) interleaved into the sync weight stream
XO0_GROUPS = [(0, 1), (1, 2), (3, 3)]


def build_nc(D=D0, H=H0, HD=HD0, SEG=SEG0, NSEG=NSEG0, mm_dt=BF16):
    """Build the per-core Bass program (same NEFF on all cores)."""
    DC = D // P                # channel chunks of 128
    L = SEG * NSEG             # tokens per core
    KC = SEG // P              # key chunks per segment
    HPC = P // HD              # heads per 128-channel chunk
    E3 = 3 * D
    HV = HD + 1                # v columns per head incl. ones column
    scale = 1.0 / math.sqrt(HD)
    assert D == H * HD and SEG % P == 0 and D % P == 0 and KC % 2 == 0

    nc = bacc.Bacc(trn_type="TRN2")
    xo_pk = nc.dram_tensor("xo_pk", [NSEG, D * SEG], mm_dt, kind="ExternalInput")
    wqk_pk = nc.dram_tensor("wqk_pk", [2 * D * D], mm_dt, kind="ExternalInput")
    wv_pk = nc.dram_tensor("wv_pk", [D * D], mm_dt, kind="ExternalInput")
    wout_pk = nc.dram_tensor("wout_pk", [D * D], mm_dt, kind="ExternalInput")
    bqkv_pt = nc.dram_tensor("bqkv_pt", [P, 3 * DC], F32, kind="ExternalInput")
    bout_pt = nc.dram_tensor("bout_pt", [P, DC], F32, kind="ExternalInput")
    bv = nc.dram_tensor("bv", [D], F32, kind="ExternalInput")
    sel2_d = nc.dram_tensor("sel2_d", [P // HD, P], mm_dt, kind="ExternalInput")
    out_pk = nc.dram_tensor("out_pk", [NSEG, D * SEG], F32, kind="ExternalOutput")
    # scratch for the softmax reciprocals: broadcast-DMA needs a DRAM source
    # (SBUF-source partition-step-0 APs are rejected)
    rc_dram = nc.dram_tensor("rc_dram", [NSEG, H * SEG], mm_dt, kind="Internal")

    def packed_ap(t, flat_off, ncols, ndc=None):
        """AP over a host-packed [P, ndc, ncols] block starting at flat_off.
        The element stream is fully sequential in DRAM."""
        if ndc is None:
            ndc = DC
        base = t[:]
        return bass.AP(tensor=base.tensor, offset=flat_off,
                       ap=[[ndc * ncols, P], [ncols, ndc], [1, ncols]])

    with ExitStack() as ctx:
        tc = ctx.enter_context(tile.TileContext(nc))
        singles = ctx.enter_context(tc.tile_pool(name="singles", bufs=1))
        xo_pool = ctx.enter_context(tc.tile_pool(name="xo", bufs=2))
        qk_pool = ctx.enter_context(tc.tile_pool(name="qk", bufs=2))
        v_pool = ctx.enter_context(tc.tile_pool(name="v", bufs=2))
        attn_pool = ctx.enter_context(tc.tile_pool(name="attn", bufs=3))
        ctxu_pool = ctx.enter_context(tc.tile_pool(name="ctxu", bufs=2))
        den_pool = ctx.enter_context(tc.tile_pool(name="den", bufs=1))
        rc_pool = ctx.enter_context(tc.tile_pool(name="rc", bufs=2))
        rows_pool = ctx.enter_context(tc.tile_pool(name="rcrows", bufs=6))
        ctxs_pool = ctx.enter_context(tc.tile_pool(name="ctxs", bufs=2))
        out_pool = ctx.enter_context(tc.tile_pool(name="outp", bufs=2))
        bcast_pool = ctx.enter_context(tc.tile_pool(name="bcast", bufs=6))
        pp_proj = ctx.enter_context(tc.tile_pool(name="pp_proj", bufs=2, space="PSUM"))
        pp_scA = ctx.enter_context(tc.tile_pool(name="pp_scA", bufs=1, space="PSUM"))
        pp_scB = ctx.enter_context(tc.tile_pool(name="pp_scB", bufs=1, space="PSUM"))
        pp_cb = ctx.enter_context(tc.tile_pool(name="pp_cb", bufs=2, space="PSUM"))

        # --- segment-0 xo halves, one per HWDGE queue (a single queue only
        # sustains ~90 GB/s, so the two descriptor streams transfer the
        # cold-start-critical bytes in parallel) ---
        xo_first = xo_pool.tile([P, DC, SEG], mm_dt, tag="xo", name="xo_s0")
        xo0_off = {}
        off = 0
        for dc0, ndc in XO0_GROUPS:
            xo0_off[dc0] = (off, ndc)
            off += P * ndc * SEG

        def xo0_dma(eng, dc0):
            o, ndc = xo0_off[dc0]
            eng.dma_start(out=xo_first[:, dc0:dc0 + ndc, :],
                          in_=packed_ap(xo_pk, o, SEG, ndc))

        xo0_dma(nc.scalar, 0)        # 128 KB: unblocks the very first MM
        xo0_dma(nc.scalar, 1)

        # tiny biases (a late bias DMA stalls the first qk-add on DVE and
        # cascades into a PE psum-WAR stall)
        bqkv_sb = singles.tile([P, 3 * DC], F32, tag="bqkv")
        nc.scalar.dma_start(out=bqkv_sb, in_=bqkv_pt[:, :])
        bout_sb = singles.tile([P, DC], F32, tag="bout")
        nc.scalar.dma_start(out=bout_sb, in_=bout_pt[:, :])
        # v-section bias broadcast along partitions (natural layout add)
        bv_sb = singles.tile([P, D], F32, tag="bv")
        bv_ap = bv[:]
        bv_bcast = bass.AP(tensor=bv_ap.tensor, offset=bv_ap.offset,
                           ap=[[0, P], *bv_ap.ap])
        nc.gpsimd.dma_start(out=bv_sb, in_=bv_bcast)

        # qk weight stream on sync, in first-use (column-group) order, with
        # the second xo0 half interleaved after the first weight group
        w_qkv_sb = singles.tile([P, DC, E3], mm_dt, tag="wqkv")
        off = 0
        for gi, (dc0, ndc, c0, nco) in enumerate(QK_GROUPS):
            nc.sync.dma_start(out=w_qkv_sb[:, dc0:dc0 + ndc, c0:c0 + nco],
                              in_=packed_ap(wqk_pk, off, nco, ndc))
            off += P * ndc * nco
            if gi == 1:
                xo0_dma(nc.sync, 3)
        off = 0
        for dc0, ndc, c0, nco in V_GROUPS:
            nc.scalar.dma_start(out=w_qkv_sb[:, dc0:dc0 + ndc,
                                             2 * D + c0:2 * D + c0 + nco],
                                in_=packed_ap(wv_pk, off, nco, ndc))
            off += P * ndc * nco
        # wout on the (otherwise idle) gpsimd queue: it is needed last and
        # this keeps the scalar queue free for xo/v-weight bytes
        w_out_sb = singles.tile([P, DC, D], mm_dt, tag="wout")
        nc.gpsimd.dma_start(out=w_out_sb[:, :, :], in_=packed_ap(wout_pk, 0, D))

        # 2x128 block-selection mask for the tail's PE-broadcast of the
        # reciprocal rows (sel2[h, m] = 1 iff m//HD == h); host-provided
        # (single-partition memsets at partition base 1 fail BIR verification)
        sel2 = singles.tile([HPC, P], mm_dt, tag="sel2")
        nc.scalar.dma_start(out=sel2, in_=sel2_d[:, :])

        def load_and_proj(s, pre_tasks=()):
            """xo load + qkv projections for segment s. pre_tasks are emitted
            after the first two qk chunks (prev segment's last recip chain --
            by then its dent DMA has landed)."""
            st = {}
            if s == 0:
                xo_s = xo_first
            else:
                xo_s = xo_pool.tile([P, DC, SEG], mm_dt, tag="xo", name=f"xo_s{s}")
                nc.scalar.dma_start(out=xo_s[:, :, :],
                                    in_=packed_ap(xo_pk, s * D * SEG, SEG))
            st["xo"] = xo_s
            st["ctxu"] = ctxu_pool.tile([P, DC, SEG], mm_dt, tag="ctxu",
                                        name=f"ctxu{s}")
            st["den"] = den_pool.tile([1, H * SEG], F32, tag="den",
                                      name=f"den{s}")
            st["ctx_s"] = ctxs_pool.tile([P, DC, SEG], mm_dt, tag="ctxs",
                                         name=f"cs{s}")
            st["out_seg"] = out_pool.tile([P, DC, SEG], F32, tag="ot",
                                          name=f"ot{s}")

            # q,k in transposed layout (e on partitions)
            qk_s = qk_pool.tile([P, 2 * DC, SEG], mm_dt, tag="qk", name=f"qk_s{s}")
            st["qk"] = qk_s
            for ec in range(2 * DC):
                ps = pp_proj.tile([P, SEG], F32, tag="proj", name=f"psqk{s}_{ec}")
                for dc in range(DC):
                    nc.tensor.matmul(
                        ps,
                        w_qkv_sb[:, dc, ec * P:(ec + 1) * P],
                        xo_s[:, dc, :],
                        start=(dc == 0), stop=(dc == DC - 1))
                nc.vector.tensor_scalar_add(qk_s[:, ec, :], ps, bqkv_sb[:, ec:ec + 1])
                if ec == 1:
                    for t in pre_tasks:
                        t()

            # v in natural layout (token on partitions), per-head + ones column
            v_s = v_pool.tile([P, KC, H * HV], mm_dt, tag="v", name=f"v_s{s}")
            st["v"] = v_s
            v_view = v_s.rearrange("p k (h c) -> p k h c", c=HV)
            nc.vector.memset(v_view[:, :, :, HD:HD + 1], 1.0)
            for lc in range(KC):
                for n0 in range(0, D, 512):
                    n = min(512, D - n0)
                    nh = n // HD
                    h0 = n0 // HD
                    psv = pp_proj.tile([P, SEG], F32, tag="proj",
                                       name=f"psv{s}_{lc}_{n0}")
                    for dc in range(DC):
                        nc.tensor.matmul(
                            psv[:, :n],
                            xo_s[:, dc, lc * P:(lc + 1) * P],
                            w_qkv_sb[:, dc, 2 * D + n0: 2 * D + n0 + n],
                            start=(dc == 0), stop=(dc == DC - 1))
                    nc.vector.tensor_add(
                        v_view[:, lc, h0:h0 + nh, 0:HD],
                        psv[:, :n].rearrange("p (h c) -> p h c", c=HD),
                        bv_sb[:, n0:n0 + n].rearrange("p (h c) -> p h c", c=HD))
            return st

        def dent_dma(s, st, hc0, nhc, tag):
            """Spread den elements for chunks [hc0,hc0+nhc) over 128
            partitions (DVE reciprocal is ~8 cyc/elem/lane; a 1-partition
            strip would take ~6us; element order is irrelevant since the
            writeback DMA restores it)."""
            den = st["den"]
            e0, ne = hc0 * HPC * SEG, nhc * HPC * SEG
            den_t = rc_pool.tile([P, ne // P], F32, tag=f"dent{tag}",
                                 name=f"dent{s}_{hc0}")
            nc.sync.dma_start(out=den_t, in_=den[0:1, e0:e0 + ne])
            st[f"dent_{hc0}_{nhc}"] = den_t

        def recip_writeback(s, st, hc0, nhc, tag, to_rows=False):
            """DVE reciprocal of the spread denominators, then either write
            back to DRAM (for the SWDGE broadcast path) or, for the tail
            chunks, a single SBUF->SBUF linearizing DMA into a [HPC, SEG]
            row tile consumed by the PE-broadcast (the SWDGE broadcast has
            ~5-7us completion latency that the tail cannot hide)."""
            den_t = st.pop(f"dent_{hc0}_{nhc}")
            e0, ne = hc0 * HPC * SEG, nhc * HPC * SEG
            rc_t = rc_pool.tile([P, ne // P], mm_dt, tag=f"rct{tag}",
                                name=f"rct{s}_{hc0}")
            with nc.allow_low_precision(
                    reason="softmax denominator reciprocal; bf16 scale factor"):
                nc.vector.reciprocal(rc_t, den_t)
            if to_rows:
                rows = rows_pool.tile([HPC, SEG], mm_dt, tag="rcrows",
                                      name=f"rcrows{s}_{hc0}")
                nc.sync.dma_start(out=rows, in_=rc_t[:, :])
                st.setdefault("rcrows", {})[hc0] = rows
            else:
                nc.sync.dma_start(out=rc_dram[s:s + 1, e0:e0 + ne], in_=rc_t)

        def attention(s, st, filler=(), per_chunk_recip=False):
            """scores + exp + unnormalized ctx (and denom), processed in
            head PAIRS: the two heads of a 128-channel chunk occupy PE
            row-groups 0-63 and 64-127, and their K=64 scores matmuls are
            emitted adjacently so the array runs them concurrently (~2x on
            the scores phase). Pipelined: ctx(pair-1) after scores(pair).
            `filler` tasks (prev segment's normalize + outproj) are emitted
            between pairs so the PE has work while ACT exp catches up.
            Returns deferred end-of-segment recip tasks for the caller to
            emit inside the next segment's projection."""
            filler = list(filler)
            n_filler = len(filler)
            emitted = 0
            qk_s, v_s = st["qk"], st["v"]
            ctxu, den = st["ctxu"], st["den"]
            ats = {}
            HH = DC // 2
            for c in range(DC + 1):
                while emitted < (c * n_filler) // DC:
                    filler[emitted]()
                    emitted += 1
                if c < DC:
                    at2 = attn_pool.tile([P, HPC, KC, SEG], mm_dt, tag="attn",
                                         name=f"at{s}_{c}")
                    ats[c] = at2
                    for w in range(KC // 2):
                        for half, pool in ((0, pp_scA), (1, pp_scB)):
                            kc = 2 * w + half
                            sc = pool.tile([P, HPC, SEG], F32, tag=f"sc{half}",
                                           name=f"sc{half}_{s}_{c}_{w}")
                            for i in range(HPC):
                                ho = i * HD
                                nc.tensor.matmul(
                                    sc[:, i, :],
                                    qk_s[ho:ho + HD, DC + c, kc * P:(kc + 1) * P],
                                    qk_s[ho:ho + HD, c, :])
                            nc.scalar.activation(
                                at2[:, :, kc, :], sc,
                                mybir.ActivationFunctionType.Exp,
                                scale=scale)
                if c > 0:
                    cp = c - 1
                    at2 = ats.pop(cp)
                    for i in range(HPC):
                        h = cp * HPC + i
                        ho = i * HD
                        cps = pp_cb.tile([P, SEG], F32, tag="cb",
                                         name=f"cps{s}_{h}")
                        for kc in range(KC):
                            nc.tensor.matmul(
                                cps[0:HD + 1, :],
                                v_s[:, kc, h * HV:(h + 1) * HV],
                                at2[:, i, kc, :],
                                start=(kc == 0), stop=(kc == KC - 1))
                        # den copy first: it is on the reciprocal-chain
                        # critical path, the ctxu cast is not
                        nc.vector.tensor_copy(den[0:1, h * SEG:(h + 1) * SEG],
                                              cps[HD:HD + 1, :])
                        nc.vector.tensor_copy(ctxu[ho:ho + HD, cp, :],
                                              cps[0:HD, :])
                    # staggered reciprocal chains: each stage is emitted a
                    # chunk after its producer so no engine-queue head ever
                    # waits long (HOL blocking)
                    if per_chunk_recip:
                        # all chunks of the final segment use the PE-broadcast
                        # rows path: SWDGE broadcast completion latency was
                        # measured at 5-16us and stalls the tail's DVE queue
                        dent_dma(s, st, cp, 1, "c")
                        if cp >= 1:
                            recip_writeback(s, st, cp - 1, 1, "c",
                                            to_rows=True)
                    else:
                        if cp == HH - 1:
                            dent_dma(s, st, 0, HH, "h")
                        elif cp == HH + 1:
                            recip_writeback(s, st, 0, HH, "h")
                            # pre-issue the SWDGE broadcasts now: their
                            # 5-16us completion latency must be paid long
                            # before the next segment's normalize fillers
                            for hc in range(HH):
                                bcast_chunk(s, st, hc)
                        elif cp == DC - 1:
                            dent_dma(s, st, HH, DC - HH, "h")
            if per_chunk_recip:
                return [lambda: recip_writeback(s, st, DC - 1, 1, "c",
                                                to_rows=True)]

            def second_half():
                recip_writeback(s, st, HH, DC - HH, "h")
                for hc in range(HH, DC):
                    bcast_chunk(s, st, hc)
            return [second_half]

        def bcast_chunk(s, st, hc):
            """Broadcast the 2 per-head reciprocal rows of chunk hc across HD
            partitions by an SWDGE DMA (partition-step-0 source AP) instead
            of a PE outer-product."""
            bcs = bcast_pool.tile([P, SEG], mm_dt, tag="bcs",
                                  name=f"bcs{s}_{hc}")
            rr = rc_dram[s:s + 1, hc * HPC * SEG:(hc + 1) * HPC * SEG]
            rr_b = bass.AP(tensor=rr.tensor, offset=rr.offset,
                           ap=[[SEG, HPC], [0, HD], [1, SEG]])
            nc.gpsimd.dma_start(out=bcs, in_=rr_b)
            st.setdefault("bcs", {})[hc] = bcs
            return bcs

        def norm_chunk(s, st, hc):
            """Normalize one 128-channel (2-head) chunk."""
            ctxu, ctx_s = st["ctxu"], st["ctx_s"]
            rows = st.get("rcrows", {}).pop(hc, None)
            if rows is not None:
                # tail path: PE outer-product broadcast (K=2 matmul against
                # the block-selection mask) instead of the slow SWDGE DMA
                bc = pp_cb.tile([P, SEG], F32, tag="cb", name=f"bcps{s}_{hc}")
                nc.tensor.matmul(bc, sel2, rows)
                nc.vector.tensor_mul(ctx_s[:, hc, :], ctxu[:, hc, :], bc)
                return
            bcs = st.get("bcs", {}).pop(hc, None)
            if bcs is None:
                bcs = bcast_chunk(s, st, hc)
                st["bcs"].pop(hc)
            nc.vector.tensor_mul(ctx_s[:, hc, :], ctxu[:, hc, :], bcs)

        def outproj_fc(s, st, fc, pool=None, tag="proj"):
            """Full out-projection column fc (all DC contraction chunks)."""
            ctx_s, out_seg = st["ctx_s"], st["out_seg"]
            pso = (pool or pp_proj).tile([P, SEG], F32, tag=tag,
                                         name=f"pso{s}_{fc}")
            for dc in range(DC):
                nc.tensor.matmul(
                    pso,
                    w_out_sb[:, dc, fc * P:(fc + 1) * P],
                    ctx_s[:, dc, :],
                    start=(dc == 0), stop=(dc == DC - 1))
            nc.vector.tensor_scalar_add(out_seg[:, fc, :], pso,
                                        bout_sb[:, fc:fc + 1])

        def store_out(s, st):
            # out_pk is fc-major [NSEG, DC, P, SEG]; this AP enumerates it
            # in the tile's (p, dc, t) order. Stores ride the scalar queue:
            # a 1.5MB store on sync would head-of-line block the tail's
            # small latency-critical DMAs.
            base = out_pk[:]
            nc.scalar.dma_start(
                out=bass.AP(tensor=base.tensor, offset=s * D * SEG,
                            ap=[[SEG, P], [P * SEG, DC], [1, SEG]]),
                in_=st["out_seg"][:, :, :])

        def store_fc(s, st, fc, eng):
            """Store one out-projection column as soon as its bias-add is
            done (the final segment's single 1.5MB store exposed ~5.5us of
            pure drain after the last matmul)."""
            base = out_pk[:]
            eng.dma_start(
                out=bass.AP(tensor=base.tensor,
                            offset=(s * DC + fc) * P * SEG,
                            ap=[[SEG, P], [1, SEG]]),
                in_=st["out_seg"][:, fc, :])

        def norm_tasks(s, st, nfc=DC, store=True):
            """Deferred normalize + outproj tasks (run as PE/DVE filler inside
            the next segment's attention). nfc/store limit how much outproj
            is spent as filler -- the rest is saved as tail fill."""
            tasks = ([(lambda hc=hc: norm_chunk(s, st, hc)) for hc in range(DC)]
                     + [(lambda fc=fc: outproj_fc(s, st, fc)) for fc in range(nfc)])
            if store:
                tasks.append(lambda: store_out(s, st))
            return tasks

        sts = {}
        SL = NSEG - 1
        deferred = []
        for s in range(NSEG):
            sts[s] = load_and_proj(s, pre_tasks=deferred)
            if s > 0:
                filler = list(norm_tasks(s - 1, sts[s - 1], nfc=4 if s == SL else DC,
                                         store=(s != SL)))
                if s != SL:
                    sts.pop(s - 1)
            else:
                filler = []
            deferred = attention(s, sts[s], filler, per_chunk_recip=(s == SL))

        # --- final segment tail. The reciprocal chains of the last two
        # chunks have ~8us of unavoidable DMA latency; the saved outproj
        # work of segment SL-1 (fc 4,5) plus a 2-bank dc-progressive for
        # SL's fc 0,1 keeps the PE busy (and HAM warm) while they drain;
        # fc 2-5 of SL run as ordinary full groups at the end. ---
        stp, stl = sts.pop(SL - 1), sts.pop(SL)
        ctx_s, out_seg = stl["ctx_s"], stl["out_seg"]
        norm_chunk(SL, stl, 0)
        norm_chunk(SL, stl, 1)
        for t in deferred:
            t()
        outproj_fc(SL - 1, stp, 4)
        outproj_fc(SL - 1, stp, 5)
        store_out(SL - 1, stp)
        n_prog = 2
        psos = {}
        for fc in range(n_prog):
            psos[fc] = pp_proj.tile([P, SEG], F32, tag="proj", name=f"psoL_{fc}")
        for dc in range(DC):
            if dc >= 2:
                norm_chunk(SL, stl, dc)
            for fc in range(n_prog):
                nc.tensor.matmul(
                    psos[fc],
                    w_out_sb[:, dc, fc * P:(fc + 1) * P],
                    ctx_s[:, dc, :],
                    start=(dc == 0), stop=(dc == DC - 1))
        for fc in range(n_prog):
            nc.vector.tensor_scalar_add(out_seg[:, fc, :], psos[fc],
                                        bout_sb[:, fc:fc + 1])
            store_fc(SL, stl, fc, nc.scalar if fc % 2 else nc.sync)
        for fc in range(n_prog, DC):
            # alternate PSUM pools: a 2-bank rotation stalls each trailing
            # group ~1.3us on the previous group's TS drain (WAR)
            outproj_fc(SL, stl, fc,
                       pool=(pp_cb if fc % 2 else pp_proj),
                       tag=("cb" if fc % 2 else "proj"))
            store_fc(SL, stl, fc, nc.scalar if fc % 2 else nc.sync)

    nc.compile()
    return nc


def _pack_w(wT, c0, ncols, dc0=0, ndc=D0 // P):
    """[D, cols] slice -> packed [P, ndc, ncols] stream (fully sequential)."""
    return np.ascontiguousarray(
        wT[:, c0:c0 + ncols].reshape(D0 // P, P, ncols)[dc0:dc0 + ndc]
        .transpose(1, 0, 2)).ravel()


def make_in_maps(x, Wqkv, bqkv, Wout, bout):
    """Shard full inputs across 8 cores: core = o*B + b."""
    r, E3, D = Wqkv.shape
    Bb, S, _ = x.shape
    DC = D // P
    in_maps = []
    for c in range(r * Bb):
        o, b = c // Bb, c % Bb
        wT = np.ascontiguousarray(Wqkv[o].T).astype(ml_dtypes.bfloat16)
        woT = np.ascontiguousarray(Wout[o].T).astype(ml_dtypes.bfloat16)
        xoT = np.ascontiguousarray(x[b, o::r, :].T).astype(ml_dtypes.bfloat16)
        wqk_pk = np.concatenate([_pack_w(wT, c0, nco, dc0, ndc)
                                 for dc0, ndc, c0, nco in QK_GROUPS])
        wv_pk = np.concatenate([_pack_w(wT, 2 * D + c0, nco, dc0, ndc)
                                for dc0, ndc, c0, nco in V_GROUPS])
        wout_pk = _pack_w(woT, 0, D)
        xo0 = np.concatenate([_pack_w(xoT, 0, SEG0, dc0, ndc)
                              for dc0, ndc in XO0_GROUPS])
        xo_pk = np.stack([xo0] + [_pack_w(xoT, s * SEG0, SEG0)
                                  for s in range(1, NSEG0)])
        in_maps.append({
            "xo_pk": xo_pk,
            "wqk_pk": wqk_pk,
            "wv_pk": wv_pk,
            "wout_pk": wout_pk,
            "bqkv_pt": np.ascontiguousarray(bqkv[o].reshape(3 * DC, P).T),
            "bout_pt": np.ascontiguousarray(bout[o].reshape(DC, P).T),
            "bv": np.ascontiguousarray(bqkv[o, 2 * D:3 * D]),
            "sel2_d": np.ascontiguousarray(
                np.kron(np.eye(P // HD0, dtype=np.float32),
                        np.ones((1, HD0), np.float32))).astype(ml_dtypes.bfloat16),
        })
    return in_maps


_NC_CACHE = {}


def get_nc():
    if "nc" not in _NC_CACHE:
        _NC_CACHE["nc"] = build_nc()
    return _NC_CACHE["nc"]


def run(inputs, trace=False, **kwargs):
    """Run the SPMD kernel; returns (full_output, BassKernelResults)."""
    x = np.ascontiguousarray(np.asarray(inputs["x"], dtype=np.float32))
    Wqkv = np.asarray(inputs["Wqkv"], dtype=np.float32)
    bqkv = np.asarray(inputs["bqkv"], dtype=np.float32)
    Wout = np.asarray(inputs["Wout"], dtype=np.float32)
    bout = np.asarray(inputs["bout"], dtype=np.float32)
    r, E3, D = Wqkv.shape
    Bb, S, _ = x.shape
    DC = D // P

    nc = get_nc()
    in_maps = make_in_maps(x, Wqkv, bqkv, Wout, bout)
    res = run_bass_kernel_spmd(nc, in_maps, core_ids=list(range(len(in_maps))),
                               trace=trace, **kwargs)

    out = np.zeros((Bb, S, r * D), np.float32)
    for c in range(len(in_maps)):
        o, b = c // Bb, c % Bb
        # unpack fc-major [NSEG, DC, P, SEG] -> [L, D] (tokens, channels)
        op = res.results[c]["out_pk"].reshape(NSEG0, DC, P, SEG0)
        oT = op.transpose(0, 3, 1, 2).reshape(S // r, D)
        out[b, o::r, o * D:(o + 1) * D] = oT
    return out, res


def kernel(x, Wqkv, bqkv, Wout, bout, num_heads):
    assert int(num_heads) == H0
    out, _ = run(dict(x=x, Wqkv=Wqkv, bqkv=bqkv, Wout=Wout, bout=bout))
    return out
